# revision 23
# baseline (speedup 1.0000x reference)
"""Trainium2 Bass kernel for nn_DenoisingNet_MLP (8-core data parallel).

Strategy: the per-patch pipeline (threshold MLPs, weight MLP, unrolled ISTA,
reconstruction) runs on device, sharded over the patch dimension: each image's
14641 patch rows are split across 2 cores (4 images x 2 = 8 cores).  The tiny
adaptive-dictionary branch (MLP on [4,112,64] + CBAM) runs on host to produce
each image's dictionary Dc; unfold/fold are host-side data movement.

Device layout is feature-major: activations are [features(part), rows(free)].
All matmuls run as float32r (TF32-like, full PE rate); the ISTA soft-threshold
sign(w)*max(|w|-l,0) is a single custom VectorE op; PSUM accumulates
w = z + y/c - (Dc^T Dc)z/c via identity/Gram matmuls.
"""
import numpy as np

import concourse.bacc as bacc
import concourse.mybir as mybir
import concourse.tile as tile
import concourse.dve_ops as dve_ops
from concourse.dve_spec import Src0, Src1, Zero, maxx, lower, Spec
from concourse.dve_uop import DveOpSpec
from concourse.bass_utils import run_bass_kernel_spmd

dt = mybir.dt
alu = mybir.AluOpType
act_f = mybir.ActivationFunctionType

# ---- problem constants (hardcoded) ----------------------------------------
N_IMG = 4
H = W = 128
PATCH = 8
T_ITER = 5
P_ROWS = 121 * 121          # 14641 patches per image
F = 512                     # max rows per sub-tile (free dim)
SUBS = [512] * 13 + [410, 256]  # per-sub-tile rows (even, >=256 for fp32r)
CS_OFF = [sum(SUBS[:i]) for i in range(len(SUBS))]
R_CORE = sum(SUBS)          # 7424 rows per core
NSUB = len(SUBS)
ODD_START = P_ROWS - R_CORE  # 7217: second core of each image starts here
N_CORES = 8

_module_cache = {}

# weight blob layout: (name, partitions, free shape)
WBLOB_SPECS = [
    ("l1_w", 64, (128,)), ("l2_w", 128, (256,)), ("l3_w", 128, (2, 128)),
    ("pd1_w", 64, (128,)), ("pd2_w", 128, (256,)), ("pd3_w", 128, (2, 128)),
    ("l4x_w", 128, (128,)), ("pd4h_w", 128, (128,)), ("pd4t_w", 128, (128,)),
    ("w1_w", 64, (256,)), ("w2_w", 128, (2, 128)), ("w3_w", 128, (64,)),
    ("dc", 64, (256,)), ("dc_lo", 64, (256,)), ("smat", 128, (2, 2, 128)),
    ("dcT", 128, (2, 64)), ("ident", 128, (128,)),
]
WBLOB_OFF = {}
_off = 0
for _nm, _p, _shp in WBLOB_SPECS:
    _flat = int(np.prod(_shp))
    WBLOB_OFF[_nm] = (_off, _flat)
    _off += _flat
WBLOB_TOT = _off
WBLOB_HOT = 640  # l1_w + l2_w + l3_w: what the first MLP chain needs


# ---- custom DVE op: soft threshold ----------------------------------------
def _soft_ref(in0, in1, s0, s1, imm2):
    return (np.sign(in0) * np.maximum(np.abs(in0) - in1, 0.0)).astype(np.float32)


def _register_soft_op():
    if "SOFT_THRESH_ANT" in dve_ops._SUB_OPCODE_FOR_NAME:
        return next(o for o in dve_ops.OPS if o.name == "SOFT_THRESH_ANT")
    s = (Src0 > Zero) - (Src0 < Zero)
    spec = Spec(body=maxx(s * Src0 - Src1, Zero) * s, reference=_soft_ref)
    shas = {}
    for ver in ("v3", "v4"):
        try:
            u = lower(spec, ver=ver)
            shas[ver] = DveOpSpec(name="SOFT_THRESH_ANT", opcode=31, uops=u,
                                  rd1_en=True).sha(ver)
        except Exception:
            pass
    op = dve_ops.DveOp("SOFT_THRESH_ANT", spec, subdim=False, uops_sha=shas)
    dve_ops.OPS.append(op)
    dve_ops.CUSTOM_DVE_SPECS[op.name] = spec
    dve_ops._SUB_OPCODE_FOR_NAME[op.name] = 31
    return op


# ---- host-side helpers ----------------------------------------------------
def _unfold(x, k, s):
    w = np.lib.stride_tricks.sliding_window_view(x[:, 0], (k, k), axis=(1, 2))
    w = w[:, ::s, ::s]
    n, ho, wo = w.shape[0], w.shape[1], w.shape[2]
    return w.reshape(n, ho * wo, k * k)


def _fold(patches, h, w, k):
    n, d, p = patches.shape
    hp = wp = h - k + 1
    pr = patches.reshape(n, k, k, hp, wp)
    out = np.zeros((n, h, w), np.float32)
    for i in range(k):
        for j in range(k):
            out[:, i:i + hp, j:j + wp] += pr[:, i, j]
    return out


def _mlp(x, p, names):
    for nm in names[:-1]:
        x = np.maximum(x @ p[nm + '_w'] + p[nm + '_b'], 0.0)
    nm = names[-1]
    return x @ p[nm + '_w'] + p[nm + '_b']


def _sigmoid(x):
    return 1.0 / (1.0 + np.exp(-x))


def _cbam(x, p):
    avg = x.mean(axis=(2, 3))
    mx = x.max(axis=(2, 3))
    mlp = lambda v: np.maximum(v @ p['ca1_w'] + p['ca1_b'], 0.0) @ p['ca2_w'] + p['ca2_b']
    ca = _sigmoid(mlp(avg) + mlp(mx))
    x = x * ca[:, :, None, None]
    s = np.stack([x.mean(axis=1), x.max(axis=1)], axis=1)
    sw = p['sa_w'][0]
    pad = np.pad(s, ((0, 0), (0, 0), (3, 3), (3, 3)))
    n = x.shape[0]
    sa = np.zeros((n, 8, 14), np.float32)
    for di in range(7):
        for dj in range(7):
            sa += pad[:, 0, di:di + 8, dj:dj + 14] * sw[0, di, dj]
            sa += pad[:, 1, di:di + 8, dj:dj + 14] * sw[1, di, dj]
    sa = _sigmoid(sa + p['sa_b'][0])
    return x * sa[:, None]


def _small_branch(x, p):
    """Adaptive dictionary branch -> atom-permuted Dc [N, 64, 256]."""
    n = x.shape[0]
    unf1 = _unfold(x, PATCH, PATCH)
    step = unf1.shape[1] // 112
    ue = unf1[:, ::step, :][:, :112, :]
    sdict = _mlp(ue, p, ['ls1', 'ls2', 'ls3', 'ls4'])
    nrm = np.maximum(np.linalg.norm(sdict, axis=-1, keepdims=True), 1e-12)
    sdict = sdict / nrm
    sdict = sdict.transpose(0, 2, 1).reshape(n, 64, 8, 14)
    sdict = _cbam(sdict, p)
    sdict = sdict.reshape(n, 64, 112)
    dc = np.concatenate(
        [np.broadcast_to(np.asarray(p['Dict']), (n, 64, 144)), sdict], axis=-1)
    # atom order: [lam 0:112 | pd 0:16 | pd 16:144]  (so the l4/pd4 heads pack
    # into two clean 128-partition tiles)
    perm = list(range(144, 256)) + list(range(0, 16)) + list(range(16, 144))
    return np.ascontiguousarray(dc[:, :, perm]).astype(np.float32)


def _to_fp32r(x):
    v = np.ascontiguousarray(x, np.float32).view(np.uint32).astype(np.uint64)
    add = 0x7FF + ((v >> 12) & 1)
    return ((v + add) & 0xFFFFF000).astype(np.uint32).view(np.float32)


def _split2(arr):
    """[256, M] -> [128, 2, M] so [:, a, :] is rows a*128 .. a*128+127."""
    m = arr.shape[1]
    return np.ascontiguousarray(arr.reshape(2, 128, m).transpose(1, 0, 2))


# ---- device module --------------------------------------------------------
def _build_module(reps=1):
    soft_op = _register_soft_op()
    nc = bacc.Bacc("TRN2")

    f32, f32r = dt.float32, dt.float32r
    din = {}
    for nm, shp, dty in [
        ("unfT", [64, R_CORE], f32r), ("unfT_lo", [64, R_CORE], f32r),
        ("wblob", [128, WBLOB_TOT], f32r),
        ("biases", [128, 14], f32),
    ]:
        din[nm] = nc.dram_tensor(nm, shp, dty, kind="ExternalInput")
    oxw = nc.dram_tensor("oxw", [64, R_CORE], f32, kind="ExternalOutput")
    owg = nc.dram_tensor("owg", [64, R_CORE], f32, kind="ExternalOutput")

    with tile.TileContext(nc) as tc:
        with tc.tile_pool(name="wts", bufs=1) as wts, \
             tc.tile_pool(name="xin", bufs=4) as xin, \
             tc.tile_pool(name="hbuf", bufs=8) as hbuf, \
             tc.tile_pool(name="h3p", bufs=3) as h3pool, \
             tc.tile_pool(name="zpool", bufs=8) as zpool, \
             tc.tile_pool(name="pair", bufs=4) as pairp, \
             tc.tile_pool(name="outs", bufs=3) as outsp, \
             tc.tile_pool(name="mp", bufs=PSUM_MP, space="PSUM") as mp, \
             tc.tile_pool(name="pp", bufs=PSUM_PP, space="PSUM") as pp:

            # load constants: a hot chunk (first chain's weights) on the HW
            # DGE queue so the first matmul starts ~4us in, the rest in
            # parallel on the gpsimd (SWDGE) queue. Separate tiles so tile
            # dependency tracking doesn't serialize readers on both DMAs.
            wt_hot = wts.tile([128, WBLOB_HOT], f32r, tag="wbhot")
            nc.sync.dma_start(wt_hot[:], din["wblob"][:, 0:WBLOB_HOT])
            wt_cold = wts.tile([128, WBLOB_TOT - WBLOB_HOT], f32r, tag="wbcold")
            nc.gpsimd.dma_start(wt_cold[:], din["wblob"][:, WBLOB_HOT:])
            wsb = {}
            for nm, part, shp in WBLOB_SPECS:
                off, flat = WBLOB_OFF[nm]
                if off < WBLOB_HOT:
                    ap = wt_hot[0:part, off:off + flat]
                else:
                    ap = wt_cold[0:part, off - WBLOB_HOT:off - WBLOB_HOT + flat]
                if len(shp) == 2:
                    ap = ap.rearrange("p (a m) -> p a m", a=shp[0])
                elif len(shp) == 3:
                    ap = ap.rearrange("p (a b m) -> p a b m", a=shp[0], b=shp[1])
                wsb[nm] = ap
            b = wts.tile([128, 14], f32, tag="biases")
            nc.gpsimd.dma_start(b[:], din["biases"][:])
            bcol = lambda j, p=128: b[0:p, j:j + 1]

            # scalar 1/c lives in a [1,1] tile; scale APs must be per-partition,
            # so instead c-folding is done host-side in biases / weights where
            # needed, and 1/c is applied via activation scale=imm below.
            # (cinv input kept for generality; value also baked into scales.)

            def relu_evict(out, psum, bias, eng):
                if eng == "act":
                    nc.scalar.activation(out, psum, act_f.Relu, bias=bias, scale=1.0)
                else:
                    nc.vector.tensor_scalar(out, psum, bias, 0.0, alu.add, alu.max)

            def chain4_gen(x_t, w1, b1, w2, b2a, b2b, w3, b3, tagp, out, eng="act"):
                """64 ->128 ->256 ->128 relu chain; leaves h3 tile in out[tagp]."""
                fs = x_t.shape[-1]
                ps1 = mp.tile([128, fs], f32, tag="mp")
                nc.tensor.matmul(ps1[:], wsb[w1][:], x_t[:], start=True, stop=True)
                h1 = hbuf.tile([128, fs], f32r, tag="h")
                relu_evict(h1[:], ps1[:], b1, eng)
                yield
                ps2a = mp.tile([128, fs], f32, tag="mp")
                nc.tensor.matmul(ps2a[:], wsb[w2][:, 0:128], h1[:], start=True, stop=True)
                h2a = hbuf.tile([128, fs], f32r, tag="h")
                relu_evict(h2a[:], ps2a[:], b2a, eng)
                ps2b = mp.tile([128, fs], f32, tag="mp")
                nc.tensor.matmul(ps2b[:], wsb[w2][:, 128:256], h1[:], start=True, stop=True)
                h2b = hbuf.tile([128, fs], f32r, tag="h")
                relu_evict(h2b[:], ps2b[:], b2b, eng)
                yield
                ps3 = mp.tile([128, fs], f32, tag="mp")
                nc.tensor.matmul(ps3[:], wsb[w3][:, 0, :], h2a[:], start=True, stop=False)
                nc.tensor.matmul(ps3[:], wsb[w3][:, 1, :], h2b[:], start=False, stop=True)
                h3 = h3pool.tile([128, fs], f32r, tag=tagp)
                relu_evict(h3[:], ps3[:], b3, eng)
                out[tagp] = h3

            def front_gen(s, st):
                """MLP chains + thresholds + wg + y + z0 for sub-tile s."""
                fs = SUBS[s]
                cs = slice(CS_OFF[s], CS_OFF[s] + fs)
                st["fs"] = fs
                x_t = xin.tile([64, fs], f32r, tag="x")
                nc.sync.dma_start(x_t[:], din["unfT"][:, cs])
                x_lo = xin.tile([64, fs], f32r, tag="xlo")
                nc.sync.dma_start(x_lo[:], din["unfT_lo"][:, cs])
                yield

                hh = {}
                yield from chain4_gen(x_t, "l1_w", bcol(0), "l2_w", bcol(1), bcol(2),
                                      "l3_w", bcol(3), "h3l", hh)
                yield
                yield from chain4_gen(x_t, "pd1_w", bcol(4), "pd2_w", bcol(5), bcol(6),
                                      "pd3_w", bcol(7), "h3p", hh, eng=PD_ENG)
                yield
                h3l, h3p = hh["h3l"], hh["h3p"]

                l_t_full = pairp.tile([128, 2, F], f32, tag="l")
                l_t = l_t_full[:, :, 0:fs]
                ps_la = mp.tile([128, fs], f32, tag="mp")
                nc.tensor.matmul(ps_la[:], wsb["l4x_w"][:], h3l[:], start=True, stop=False)
                nc.tensor.matmul(ps_la[:], wsb["pd4h_w"][:], h3p[:], start=False, stop=True)
                nc.scalar.activation(l_t[:, 0, :], ps_la[:], act_f.Identity,
                                     bias=bcol(12), scale=CINV)
                ps_lb = mp.tile([128, fs], f32, tag="mp")
                nc.tensor.matmul(ps_lb[:], wsb["pd4t_w"][:], h3p[:], start=True, stop=True)
                nc.scalar.activation(l_t[:, 1, :], ps_lb[:], act_f.Identity,
                                     bias=bcol(13), scale=CINV)
                yield

                psw1a = mp.tile([128, fs], f32, tag="mp")
                nc.tensor.matmul(psw1a[:], wsb["w1_w"][:, 0:128], x_t[:], start=True, stop=True)
                wh1a = hbuf.tile([128, fs], f32r, tag="h")
                nc.scalar.activation(wh1a[:], psw1a[:], act_f.Relu, bias=bcol(8), scale=1.0)
                psw1b = mp.tile([128, fs], f32, tag="mp")
                nc.tensor.matmul(psw1b[:], wsb["w1_w"][:, 128:256], x_t[:], start=True, stop=True)
                wh1b = hbuf.tile([128, fs], f32r, tag="h")
                relu_evict(wh1b[:], psw1b[:], bcol(9), W_ENG)
                yield
                psw2 = mp.tile([128, fs], f32, tag="mp")
                nc.tensor.matmul(psw2[:], wsb["w2_w"][:, 0, :], wh1a[:], start=True, stop=False)
                nc.tensor.matmul(psw2[:], wsb["w2_w"][:, 1, :], wh1b[:], start=False, stop=True)
                wh2 = hbuf.tile([128, fs], f32r, tag="h")
                relu_evict(wh2[:], psw2[:], bcol(10), W_ENG)
                psw3 = mp.tile([64, fs], f32, tag="mp")
                nc.tensor.matmul(psw3[:], wsb["w3_w"][:], wh2[:], start=True, stop=True)
                wg = outsp.tile([64, fs], f32, tag="wg")
                nc.scalar.activation(wg[:], psw3[:], act_f.Sigmoid,
                                     bias=bcol(11, 64), scale=1.0)
                yield

                ps_y_full = pp.tile([128, 2, F], f32, tag="pp")
                ps_y = ps_y_full[:, :, 0:fs]
                for k in (0, 1):
                    ck = slice(k * 128, (k + 1) * 128)
                    nc.tensor.matmul(ps_y[:, k, :], wsb["dc"][:, ck], x_t[:], start=True, stop=False)
                    nc.tensor.matmul(ps_y[:, k, :], wsb["dc"][:, ck], x_lo[:], start=False, stop=False)
                    nc.tensor.matmul(ps_y[:, k, :], wsb["dc_lo"][:, ck], x_t[:], start=False, stop=True)
                yc_full = pairp.tile([128, 2, F], f32r, tag="yc")
                yc = yc_full[:, :, 0:fs]
                nc.scalar.mul(yc[:], ps_y[:], CINV)
                z_full = zpool.tile([128, 2, F], f32r, tag="z")
                z = z_full[:, :, 0:fs]
                nc.vector._custom_dve(soft_op, out=z[:], in0=ps_y[:], in1=l_t[:])
                st.update({"s": s, "z": z, "yc": yc, "l": l_t, "wg": wg})

            def emit_iter_gen(st):
                fs = st["fs"]
                ps_w_full = pp.tile([128, 2, F], f32, tag="pp")
                ps_w = ps_w_full[:, :, 0:fs]
                z, yc = st["z"], st["yc"]
                for k in (0, 1):
                    nc.tensor.matmul(ps_w[:, k, :], wsb["ident"][:],
                                     yc[:, k, :], start=True, stop=False)
                    nc.tensor.matmul(ps_w[:, k, :], wsb["smat"][:, 0, k, :],
                                     z[:, 0, :], start=False, stop=False)
                    nc.tensor.matmul(ps_w[:, k, :], wsb["smat"][:, 1, k, :],
                                     z[:, 1, :], start=False, stop=True)
                    if k == 0:
                        yield
                zn_full = zpool.tile([128, 2, F], f32r, tag="z")
                zn = zn_full[:, :, 0:fs]
                nc.vector._custom_dve(soft_op, out=zn[:], in0=ps_w[:], in1=st["l"])
                st["z"] = zn

            def emit_back(st):
                s, z, wg, fs = st["s"], st["z"], st["wg"], st["fs"]
                cs = slice(CS_OFF[s], CS_OFF[s] + fs)
                ps_xp = mp.tile([64, fs], f32, tag="mp")
                nc.tensor.matmul(ps_xp[:], wsb["dcT"][:, 0, :], z[:, 0, :], start=True, stop=False)
                nc.tensor.matmul(ps_xp[:], wsb["dcT"][:, 1, :], z[:, 1, :], start=False, stop=True)
                xpc = outsp.tile([64, fs], f32, tag="xpc")
                nc.vector.tensor_scalar(xpc[:], ps_xp[:], 0.0, 1.0, alu.max, alu.min)
                xw = outsp.tile([64, fs], f32, tag="xw")
                nc.vector.tensor_tensor(xw[:], xpc[:], wg[:], alu.mult)
                nc.sync.dma_start(oxw[:, cs], xw[:])
                nc.sync.dma_start(owg[:, cs], wg[:])

            def ista_gen(st):
                for _t in range(T_ITER):
                    yield from emit_iter_gen(st)
                    yield
                emit_back(st)

            # software pipeline: front(s) interleaved with the pending ISTA
            # chains of previous sub-tiles so PE stays dense (HAM warm)
            # through the ACT-gated MLP chain stages.
            for _rep in range(reps):
                from collections import deque
                pend = deque()
                rr = 0
                for s in range(NSUB):
                    st = {}
                    fg = front_gen(s, st)
                    k = 0
                    for _ in fg:
                        k += 1
                        if pend and k % ILV == 0:
                            g = pend[rr % len(pend)]
                            if next(g, "done") == "done":
                                pend.remove(g)
                            rr += 1
                    while len(pend) >= PIPE_DEPTH:
                        g = pend[0]
                        if next(g, "done") == "done":
                            pend.popleft()
                    pend.append(ista_gen(st))
                while pend:
                    g = pend[0]
                    if next(g, "done") == "done":
                        pend.popleft()

    nc.finalize()
    return nc


CINV = None  # set before _build_module is called (compile-time constant)


def _get_module(c, reps=1):
    global CINV
    key = (round(1.0 / c, 10), reps)
    if key not in _module_cache:
        CINV = 1.0 / c
        _module_cache[key] = _build_module(reps=reps)
    return _module_cache[key]


# ---- public entry ----------------------------------------------------------
def kernel(x, params, _reps=1):
    x = np.asarray(x, np.float32)
    p = {k: np.asarray(v, np.float32) for k, v in params.items()}
    c = float(p['c'])

    dcp = _small_branch(x, p)                   # [N, 64, 256] permuted atoms
    unf = _unfold(x, PATCH, 1)                  # [N, P, 64]

    # shared (replicated) tensors
    l4x_w = np.concatenate([p['l4_w'], np.zeros((128, 16), np.float32)], axis=1)
    pd4h_w = np.concatenate([np.zeros((128, 112), np.float32), p['pd4_w'][:, :16]], axis=1)
    pd4t_w = np.ascontiguousarray(p['pd4_w'][:, 16:144])
    biases = np.zeros((128, 14), np.float32)
    biases[:, 0] = p['l1_b']
    biases[:, 1] = p['l2_b'][:128]
    biases[:, 2] = p['l2_b'][128:]
    biases[:, 3] = p['l3_b']
    biases[:, 4] = p['pd1_b']
    biases[:, 5] = p['pd2_b'][:128]
    biases[:, 6] = p['pd2_b'][128:]
    biases[:, 7] = p['pd3_b']
    biases[:, 8] = p['w1_b'][:128]
    biases[:, 9] = p['w1_b'][128:]
    biases[:, 10] = p['w2_b']
    biases[0:64, 11] = p['w3_b']
    biases[:, 12] = np.concatenate([p['l4_b'][:112], p['pd4_b'][:16]]) / c
    biases[:, 13] = p['pd4_b'][16:144] / c

    shared = {
        "l1_w": p['l1_w'], "l2_w": p['l2_w'], "l3_w": _split2(p['l3_w']),
        "pd1_w": p['pd1_w'], "pd2_w": p['pd2_w'], "pd3_w": _split2(p['pd3_w']),
        "l4x_w": l4x_w, "pd4h_w": pd4h_w, "pd4t_w": pd4t_w,
        "w1_w": p['w1_w'], "w2_w": _split2(p['w2_w']),
        "w3_w": p['w3_w'],
        "ident": np.eye(128, dtype=np.float32),
    }

    def build_blob(tensors):
        blob = np.zeros((128, WBLOB_TOT), np.float32)
        for nm, part, shp in WBLOB_SPECS:
            off, flat = WBLOB_OFF[nm]
            blob[0:part, off:off + flat] = tensors[nm].reshape(part, flat)
        return blob

    # per-image dictionary tensors
    per_img = []
    for n in range(N_IMG):
        dc = dcp[n]                                       # [64, 256]
        dc_hi = _to_fp32r(dc)
        smat = (np.eye(256, dtype=np.float64)
                - (dc.astype(np.float64).T @ dc.astype(np.float64)) / c).astype(np.float32)
        per_img.append({
            "dc": dc_hi,
            "dc_lo": (dc - dc_hi).astype(np.float32),
            "smat": np.ascontiguousarray(
                smat.reshape(2, 128, 2, 128).transpose(1, 0, 2, 3)),
            "dcT": _split2(np.ascontiguousarray(dc.T)),
        })

    blobs = [build_blob({**shared, **per_img[n]}) for n in range(N_IMG)]
    in_maps = []
    for core in range(N_CORES):
        n, half = core // 2, core % 2
        start = 0 if half == 0 else ODD_START
        unfT = np.ascontiguousarray(unf[n][start:start + R_CORE].T)  # [64, R_CORE]
        unfT_hi = _to_fp32r(unfT)
        in_maps.append({"unfT": unfT_hi,
                        "unfT_lo": (unfT - unfT_hi).astype(np.float32),
                        "wblob": blobs[n], "biases": biases})

    nc = _get_module(c, reps=_reps)
    res = run_bass_kernel_spmd(nc, in_maps, list(range(N_CORES)), trace=False)

    # reassemble full [N, 64, P] outputs (discard overlap region duplicates)
    xw_full = np.zeros((N_IMG, 64, P_ROWS), np.float32)
    wg_full = np.zeros((N_IMG, 64, P_ROWS), np.float32)
    for core in range(N_CORES):
        n, half = core // 2, core % 2
        r = res.results[core]
        keep = (P_ROWS + 1) // 2          # 7321 rows from the even core
        if half == 0:
            xw_full[n, :, :keep] = r["oxw"][:, :keep]
            wg_full[n, :, :keep] = r["owg"][:, :keep]
        else:
            xw_full[n, :, keep:] = r["oxw"][:, keep - ODD_START:]
            wg_full[n, :, keep:] = r["owg"][:, keep - ODD_START:]

    fx = _fold(xw_full, H, W, PATCH)
    fn = _fold(wg_full, H, W, PATCH)
    return (fx / fn)[:, None].astype(np.float32)


# revision 24
# speedup vs baseline: 1.0124x; 1.0124x over previous
"""Trainium2 Bass kernel for nn_DenoisingNet_MLP (8-core data parallel).

Strategy: the per-patch pipeline (threshold MLPs, weight MLP, unrolled ISTA,
reconstruction) runs on device, sharded over the patch dimension: each image's
14641 patch rows are split across 2 cores (4 images x 2 = 8 cores).  The tiny
adaptive-dictionary branch (MLP on [4,112,64] + CBAM) runs on host to produce
each image's dictionary Dc; unfold/fold are host-side data movement.

Device layout is feature-major: activations are [features(part), rows(free)].
All matmuls run as float32r (TF32-like, full PE rate); the ISTA soft-threshold
sign(w)*max(|w|-l,0) is a single custom VectorE op; PSUM accumulates
w = z + y/c - (Dc^T Dc)z/c via identity/Gram matmuls.
"""
import numpy as np

import concourse.bacc as bacc
import concourse.mybir as mybir
import concourse.tile as tile
import concourse.dve_ops as dve_ops
from concourse.dve_spec import Src0, Src1, Zero, maxx, lower, Spec
from concourse.dve_uop import DveOpSpec
from concourse.bass_utils import run_bass_kernel_spmd

dt = mybir.dt
alu = mybir.AluOpType
act_f = mybir.ActivationFunctionType

# ---- problem constants (hardcoded) ----------------------------------------
N_IMG = 4
H = W = 128
PATCH = 8
T_ITER = 5
P_ROWS = 121 * 121          # 14641 patches per image
F = 512                     # max rows per sub-tile (free dim)
SUBS = [512] * 13 + [410, 256]  # per-sub-tile rows (even, >=256 for fp32r)
CS_OFF = [sum(SUBS[:i]) for i in range(len(SUBS))]
R_CORE = sum(SUBS)          # 7424 rows per core
NSUB = len(SUBS)
ODD_START = P_ROWS - R_CORE  # 7217: second core of each image starts here
N_CORES = 8

_module_cache = {}

# weight blob layout: (name, partitions, free shape)
WBLOB_SPECS = [
    ("l1_w", 64, (128,)), ("l2_w", 128, (256,)), ("l3_w", 128, (2, 128)),
    ("pd1_w", 64, (128,)), ("pd2_w", 128, (256,)), ("pd3_w", 128, (2, 128)),
    ("l4x_w", 128, (128,)), ("pd4h_w", 128, (128,)), ("pd4t_w", 128, (128,)),
    ("w1_w", 64, (256,)), ("w2_w", 128, (2, 128)), ("w3_w", 128, (64,)),
    ("dc", 64, (256,)), ("dc_lo", 64, (256,)), ("smat", 128, (2, 2, 128)),
    ("dcT", 128, (2, 64)), ("ident", 128, (128,)),
]
WBLOB_OFF = {}
_off = 0
for _nm, _p, _shp in WBLOB_SPECS:
    _flat = int(np.prod(_shp))
    WBLOB_OFF[_nm] = (_off, _flat)
    _off += _flat
WBLOB_TOT = _off
WBLOB_HOT = 640  # l1_w + l2_w + l3_w: what the first MLP chain needs


# ---- custom DVE op: soft threshold ----------------------------------------
def _soft_ref(in0, in1, s0, s1, imm2):
    return (np.sign(in0) * np.maximum(np.abs(in0) - in1, 0.0)).astype(np.float32)


def _register_soft_op():
    if "SOFT_THRESH_ANT" in dve_ops._SUB_OPCODE_FOR_NAME:
        return next(o for o in dve_ops.OPS if o.name == "SOFT_THRESH_ANT")
    s = (Src0 > Zero) - (Src0 < Zero)
    spec = Spec(body=maxx(s * Src0 - Src1, Zero) * s, reference=_soft_ref)
    shas = {}
    for ver in ("v3", "v4"):
        try:
            u = lower(spec, ver=ver)
            shas[ver] = DveOpSpec(name="SOFT_THRESH_ANT", opcode=31, uops=u,
                                  rd1_en=True).sha(ver)
        except Exception:
            pass
    op = dve_ops.DveOp("SOFT_THRESH_ANT", spec, subdim=False, uops_sha=shas)
    dve_ops.OPS.append(op)
    dve_ops.CUSTOM_DVE_SPECS[op.name] = spec
    dve_ops._SUB_OPCODE_FOR_NAME[op.name] = 31
    return op


# ---- host-side helpers ----------------------------------------------------
def _unfold(x, k, s):
    w = np.lib.stride_tricks.sliding_window_view(x[:, 0], (k, k), axis=(1, 2))
    w = w[:, ::s, ::s]
    n, ho, wo = w.shape[0], w.shape[1], w.shape[2]
    return w.reshape(n, ho * wo, k * k)


def _fold(patches, h, w, k):
    n, d, p = patches.shape
    hp = wp = h - k + 1
    pr = patches.reshape(n, k, k, hp, wp)
    out = np.zeros((n, h, w), np.float32)
    for i in range(k):
        for j in range(k):
            out[:, i:i + hp, j:j + wp] += pr[:, i, j]
    return out


def _mlp(x, p, names):
    for nm in names[:-1]:
        x = np.maximum(x @ p[nm + '_w'] + p[nm + '_b'], 0.0)
    nm = names[-1]
    return x @ p[nm + '_w'] + p[nm + '_b']


def _sigmoid(x):
    return 1.0 / (1.0 + np.exp(-x))


def _cbam(x, p):
    avg = x.mean(axis=(2, 3))
    mx = x.max(axis=(2, 3))
    mlp = lambda v: np.maximum(v @ p['ca1_w'] + p['ca1_b'], 0.0) @ p['ca2_w'] + p['ca2_b']
    ca = _sigmoid(mlp(avg) + mlp(mx))
    x = x * ca[:, :, None, None]
    s = np.stack([x.mean(axis=1), x.max(axis=1)], axis=1)
    sw = p['sa_w'][0]
    pad = np.pad(s, ((0, 0), (0, 0), (3, 3), (3, 3)))
    n = x.shape[0]
    sa = np.zeros((n, 8, 14), np.float32)
    for di in range(7):
        for dj in range(7):
            sa += pad[:, 0, di:di + 8, dj:dj + 14] * sw[0, di, dj]
            sa += pad[:, 1, di:di + 8, dj:dj + 14] * sw[1, di, dj]
    sa = _sigmoid(sa + p['sa_b'][0])
    return x * sa[:, None]


def _small_branch(x, p):
    """Adaptive dictionary branch -> atom-permuted Dc [N, 64, 256]."""
    n = x.shape[0]
    unf1 = _unfold(x, PATCH, PATCH)
    step = unf1.shape[1] // 112
    ue = unf1[:, ::step, :][:, :112, :]
    sdict = _mlp(ue, p, ['ls1', 'ls2', 'ls3', 'ls4'])
    nrm = np.maximum(np.linalg.norm(sdict, axis=-1, keepdims=True), 1e-12)
    sdict = sdict / nrm
    sdict = sdict.transpose(0, 2, 1).reshape(n, 64, 8, 14)
    sdict = _cbam(sdict, p)
    sdict = sdict.reshape(n, 64, 112)
    dc = np.concatenate(
        [np.broadcast_to(np.asarray(p['Dict']), (n, 64, 144)), sdict], axis=-1)
    # atom order: [lam 0:112 | pd 0:16 | pd 16:144]  (so the l4/pd4 heads pack
    # into two clean 128-partition tiles)
    perm = list(range(144, 256)) + list(range(0, 16)) + list(range(16, 144))
    return np.ascontiguousarray(dc[:, :, perm]).astype(np.float32)


def _to_fp32r(x):
    v = np.ascontiguousarray(x, np.float32).view(np.uint32).astype(np.uint64)
    add = 0x7FF + ((v >> 12) & 1)
    return ((v + add) & 0xFFFFF000).astype(np.uint32).view(np.float32)


def _split2(arr):
    """[256, M] -> [128, 2, M] so [:, a, :] is rows a*128 .. a*128+127."""
    m = arr.shape[1]
    return np.ascontiguousarray(arr.reshape(2, 128, m).transpose(1, 0, 2))


# ---- device module --------------------------------------------------------
def _build_module(reps=1):
    soft_op = _register_soft_op()
    nc = bacc.Bacc("TRN2")

    f32, f32r = dt.float32, dt.float32r
    din = {}
    for nm, shp, dty in [
        ("unfT", [64, R_CORE], f32r), ("unfT_lo", [64, R_CORE], f32r),
        ("wblob", [128, WBLOB_TOT], f32r),
        ("biases", [128, 14], f32),
    ]:
        din[nm] = nc.dram_tensor(nm, shp, dty, kind="ExternalInput")
    oxw = nc.dram_tensor("oxw", [64, R_CORE], f32, kind="ExternalOutput")
    owg = nc.dram_tensor("owg", [64, R_CORE], f32, kind="ExternalOutput")

    with tile.TileContext(nc) as tc:
        with tc.tile_pool(name="wts", bufs=1) as wts, \
             tc.tile_pool(name="xin", bufs=4) as xin, \
             tc.tile_pool(name="hbuf", bufs=8) as hbuf, \
             tc.tile_pool(name="h3p", bufs=3) as h3pool, \
             tc.tile_pool(name="zpool", bufs=8) as zpool, \
             tc.tile_pool(name="pair", bufs=4) as pairp, \
             tc.tile_pool(name="outs", bufs=3) as outsp, \
             tc.tile_pool(name="mp", bufs=PSUM_MP, space="PSUM") as mp, \
             tc.tile_pool(name="pp", bufs=PSUM_PP, space="PSUM") as pp:

            # load constants: a hot chunk (first chain's weights) on the HW
            # DGE queue so the first matmul starts ~4us in, the rest in
            # parallel on the gpsimd (SWDGE) queue. Separate tiles so tile
            # dependency tracking doesn't serialize readers on both DMAs.
            wt_hot = wts.tile([128, WBLOB_HOT], f32r, tag="wbhot")
            nc.sync.dma_start(wt_hot[:], din["wblob"][:, 0:WBLOB_HOT])
            wt_cold = wts.tile([128, WBLOB_TOT - WBLOB_HOT], f32r, tag="wbcold")
            nc.gpsimd.dma_start(wt_cold[:], din["wblob"][:, WBLOB_HOT:])
            wsb = {}
            for nm, part, shp in WBLOB_SPECS:
                off, flat = WBLOB_OFF[nm]
                if off < WBLOB_HOT:
                    ap = wt_hot[0:part, off:off + flat]
                else:
                    ap = wt_cold[0:part, off - WBLOB_HOT:off - WBLOB_HOT + flat]
                if len(shp) == 2:
                    ap = ap.rearrange("p (a m) -> p a m", a=shp[0])
                elif len(shp) == 3:
                    ap = ap.rearrange("p (a b m) -> p a b m", a=shp[0], b=shp[1])
                wsb[nm] = ap
            b = wts.tile([128, 14], f32, tag="biases")
            nc.gpsimd.dma_start(b[:], din["biases"][:])
            bcol = lambda j, p=128: b[0:p, j:j + 1]

            # scalar 1/c lives in a [1,1] tile; scale APs must be per-partition,
            # so instead c-folding is done host-side in biases / weights where
            # needed, and 1/c is applied via activation scale=imm below.
            # (cinv input kept for generality; value also baked into scales.)

            def relu_evict(out, psum, bias, eng):
                if eng == "act":
                    nc.scalar.activation(out, psum, act_f.Relu, bias=bias, scale=1.0)
                else:
                    nc.vector.tensor_scalar(out, psum, bias, 0.0, alu.add, alu.max)

            def chain4_gen(x_t, w1, b1, w2, b2a, b2b, w3, b3, tagp, out, eng="act"):
                """64 ->128 ->256 ->128 relu chain; leaves h3 tile in out[tagp]."""
                fs = x_t.shape[-1]
                ps1 = mp.tile([128, fs], f32, tag="mp")
                nc.tensor.matmul(ps1[:], wsb[w1][:], x_t[:], start=True, stop=True)
                h1 = hbuf.tile([128, fs], f32r, tag="h")
                relu_evict(h1[:], ps1[:], b1, eng)
                yield
                ps2a = mp.tile([128, fs], f32, tag="mp")
                nc.tensor.matmul(ps2a[:], wsb[w2][:, 0:128], h1[:], start=True, stop=True)
                h2a = hbuf.tile([128, fs], f32r, tag="h")
                relu_evict(h2a[:], ps2a[:], b2a, eng)
                ps2b = mp.tile([128, fs], f32, tag="mp")
                nc.tensor.matmul(ps2b[:], wsb[w2][:, 128:256], h1[:], start=True, stop=True)
                h2b = hbuf.tile([128, fs], f32r, tag="h")
                relu_evict(h2b[:], ps2b[:], b2b, eng)
                yield
                ps3 = mp.tile([128, fs], f32, tag="mp")
                nc.tensor.matmul(ps3[:], wsb[w3][:, 0, :], h2a[:], start=True, stop=False)
                nc.tensor.matmul(ps3[:], wsb[w3][:, 1, :], h2b[:], start=False, stop=True)
                h3 = h3pool.tile([128, fs], f32r, tag=tagp)
                relu_evict(h3[:], ps3[:], b3, eng)
                out[tagp] = h3

            def front_gen(s, st):
                """MLP chains + thresholds + wg + y + z0 for sub-tile s."""
                fs = SUBS[s]
                cs = slice(CS_OFF[s], CS_OFF[s] + fs)
                st["fs"] = fs
                x_t = xin.tile([64, fs], f32r, tag="x")
                nc.sync.dma_start(x_t[:], din["unfT"][:, cs])
                x_lo = xin.tile([64, fs], f32r, tag="xlo")
                nc.sync.dma_start(x_lo[:], din["unfT_lo"][:, cs])
                yield

                hh = {}
                yield from chain4_gen(x_t, "l1_w", bcol(0), "l2_w", bcol(1), bcol(2),
                                      "l3_w", bcol(3), "h3l", hh)
                yield
                yield from chain4_gen(x_t, "pd1_w", bcol(4), "pd2_w", bcol(5), bcol(6),
                                      "pd3_w", bcol(7), "h3p", hh, eng=PD_ENG)
                yield
                h3l, h3p = hh["h3l"], hh["h3p"]

                l_t_full = pairp.tile([128, 2, F], f32, tag="l")
                l_t = l_t_full[:, :, 0:fs]
                ps_la = mp.tile([128, fs], f32, tag="mp")
                nc.tensor.matmul(ps_la[:], wsb["l4x_w"][:], h3l[:], start=True, stop=False)
                nc.tensor.matmul(ps_la[:], wsb["pd4h_w"][:], h3p[:], start=False, stop=True)
                nc.scalar.activation(l_t[:, 0, :], ps_la[:], act_f.Identity,
                                     bias=bcol(12), scale=CINV)
                ps_lb = mp.tile([128, fs], f32, tag="mp")
                nc.tensor.matmul(ps_lb[:], wsb["pd4t_w"][:], h3p[:], start=True, stop=True)
                nc.scalar.activation(l_t[:, 1, :], ps_lb[:], act_f.Identity,
                                     bias=bcol(13), scale=CINV)
                yield

                psw1a = mp.tile([128, fs], f32, tag="mp")
                nc.tensor.matmul(psw1a[:], wsb["w1_w"][:, 0:128], x_t[:], start=True, stop=True)
                wh1a = hbuf.tile([128, fs], f32r, tag="h")
                nc.scalar.activation(wh1a[:], psw1a[:], act_f.Relu, bias=bcol(8), scale=1.0)
                psw1b = mp.tile([128, fs], f32, tag="mp")
                nc.tensor.matmul(psw1b[:], wsb["w1_w"][:, 128:256], x_t[:], start=True, stop=True)
                wh1b = hbuf.tile([128, fs], f32r, tag="h")
                relu_evict(wh1b[:], psw1b[:], bcol(9), W_ENG)
                yield
                psw2 = mp.tile([128, fs], f32, tag="mp")
                nc.tensor.matmul(psw2[:], wsb["w2_w"][:, 0, :], wh1a[:], start=True, stop=False)
                nc.tensor.matmul(psw2[:], wsb["w2_w"][:, 1, :], wh1b[:], start=False, stop=True)
                wh2 = hbuf.tile([128, fs], f32r, tag="h")
                relu_evict(wh2[:], psw2[:], bcol(10), W_ENG)
                psw3 = mp.tile([64, fs], f32, tag="mp")
                nc.tensor.matmul(psw3[:], wsb["w3_w"][:], wh2[:], start=True, stop=True)
                wg = outsp.tile([64, fs], f32, tag="wg")
                nc.scalar.activation(wg[:], psw3[:], act_f.Sigmoid,
                                     bias=bcol(11, 64), scale=1.0)
                yield

                ps_y_full = pp.tile([128, 2, F], f32, tag="pp")
                ps_y = ps_y_full[:, :, 0:fs]
                for k in (0, 1):
                    ck = slice(k * 128, (k + 1) * 128)
                    nc.tensor.matmul(ps_y[:, k, :], wsb["dc"][:, ck], x_t[:], start=True, stop=False)
                    nc.tensor.matmul(ps_y[:, k, :], wsb["dc"][:, ck], x_lo[:], start=False, stop=False)
                    nc.tensor.matmul(ps_y[:, k, :], wsb["dc_lo"][:, ck], x_t[:], start=False, stop=True)
                yc_full = pairp.tile([128, 2, F], f32r, tag="yc")
                yc = yc_full[:, :, 0:fs]
                nc.scalar.mul(yc[:], ps_y[:], CINV)
                z_full = zpool.tile([128, 2, F], f32r, tag="z")
                z = z_full[:, :, 0:fs]
                nc.vector._custom_dve(soft_op, out=z[:], in0=ps_y[:], in1=l_t[:])
                st.update({"s": s, "z": z, "yc": yc, "l": l_t, "wg": wg})

            def emit_iter_gen(st):
                fs = st["fs"]
                ps_w_full = pp.tile([128, 2, F], f32, tag="pp")
                ps_w = ps_w_full[:, :, 0:fs]
                z, yc = st["z"], st["yc"]
                for k in (0, 1):
                    nc.tensor.matmul(ps_w[:, k, :], wsb["ident"][:],
                                     yc[:, k, :], start=True, stop=False)
                    nc.tensor.matmul(ps_w[:, k, :], wsb["smat"][:, 0, k, :],
                                     z[:, 0, :], start=False, stop=False)
                    nc.tensor.matmul(ps_w[:, k, :], wsb["smat"][:, 1, k, :],
                                     z[:, 1, :], start=False, stop=True)
                    if k == 0:
                        yield
                zn_full = zpool.tile([128, 2, F], f32r, tag="z")
                zn = zn_full[:, :, 0:fs]
                nc.vector._custom_dve(soft_op, out=zn[:], in0=ps_w[:], in1=st["l"])
                st["z"] = zn

            def emit_back(st):
                s, z, wg, fs = st["s"], st["z"], st["wg"], st["fs"]
                cs = slice(CS_OFF[s], CS_OFF[s] + fs)
                ps_xp = mp.tile([64, fs], f32, tag="mp")
                nc.tensor.matmul(ps_xp[:], wsb["dcT"][:, 0, :], z[:, 0, :], start=True, stop=False)
                nc.tensor.matmul(ps_xp[:], wsb["dcT"][:, 1, :], z[:, 1, :], start=False, stop=True)
                xpc = outsp.tile([64, fs], f32, tag="xpc")
                nc.vector.tensor_scalar(xpc[:], ps_xp[:], 0.0, 1.0, alu.max, alu.min)
                xw = outsp.tile([64, fs], f32, tag="xw")
                nc.vector.tensor_tensor(xw[:], xpc[:], wg[:], alu.mult)
                nc.sync.dma_start(oxw[:, cs], xw[:])
                nc.sync.dma_start(owg[:, cs], wg[:])

            def ista_gen(st):
                for _t in range(T_ITER):
                    yield from emit_iter_gen(st)
                    yield
                emit_back(st)

            # software pipeline: front(s) interleaved with the pending ISTA
            # chains of previous sub-tiles so PE stays dense (HAM warm)
            # through the ACT-gated MLP chain stages.
            for _rep in range(reps):
                from collections import deque
                pend = deque()
                rr = 0
                for s in range(NSUB):
                    st = {}
                    fg = front_gen(s, st)
                    k = 0
                    for _ in fg:
                        k += 1
                        if pend and k % ILV == 0:
                            g = pend[rr % len(pend)]
                            if next(g, "done") == "done":
                                pend.remove(g)
                            rr += 1
                    while len(pend) >= PIPE_DEPTH:
                        g = pend[0]
                        if next(g, "done") == "done":
                            pend.popleft()
                    pend.append(ista_gen(st))
                while pend:
                    g = pend[rr % len(pend)]
                    if next(g, "done") == "done":
                        pend.remove(g)
                    rr += 1

    nc.finalize()
    return nc


CINV = None  # set before _build_module is called (compile-time constant)


def _get_module(c, reps=1):
    global CINV
    key = (round(1.0 / c, 10), reps)
    if key not in _module_cache:
        CINV = 1.0 / c
        _module_cache[key] = _build_module(reps=reps)
    return _module_cache[key]


# ---- public entry ----------------------------------------------------------
def kernel(x, params, _reps=1):
    x = np.asarray(x, np.float32)
    p = {k: np.asarray(v, np.float32) for k, v in params.items()}
    c = float(p['c'])

    dcp = _small_branch(x, p)                   # [N, 64, 256] permuted atoms
    unf = _unfold(x, PATCH, 1)                  # [N, P, 64]

    # shared (replicated) tensors
    l4x_w = np.concatenate([p['l4_w'], np.zeros((128, 16), np.float32)], axis=1)
    pd4h_w = np.concatenate([np.zeros((128, 112), np.float32), p['pd4_w'][:, :16]], axis=1)
    pd4t_w = np.ascontiguousarray(p['pd4_w'][:, 16:144])
    biases = np.zeros((128, 14), np.float32)
    biases[:, 0] = p['l1_b']
    biases[:, 1] = p['l2_b'][:128]
    biases[:, 2] = p['l2_b'][128:]
    biases[:, 3] = p['l3_b']
    biases[:, 4] = p['pd1_b']
    biases[:, 5] = p['pd2_b'][:128]
    biases[:, 6] = p['pd2_b'][128:]
    biases[:, 7] = p['pd3_b']
    biases[:, 8] = p['w1_b'][:128]
    biases[:, 9] = p['w1_b'][128:]
    biases[:, 10] = p['w2_b']
    biases[0:64, 11] = p['w3_b']
    biases[:, 12] = np.concatenate([p['l4_b'][:112], p['pd4_b'][:16]]) / c
    biases[:, 13] = p['pd4_b'][16:144] / c

    shared = {
        "l1_w": p['l1_w'], "l2_w": p['l2_w'], "l3_w": _split2(p['l3_w']),
        "pd1_w": p['pd1_w'], "pd2_w": p['pd2_w'], "pd3_w": _split2(p['pd3_w']),
        "l4x_w": l4x_w, "pd4h_w": pd4h_w, "pd4t_w": pd4t_w,
        "w1_w": p['w1_w'], "w2_w": _split2(p['w2_w']),
        "w3_w": p['w3_w'],
        "ident": np.eye(128, dtype=np.float32),
    }

    def build_blob(tensors):
        blob = np.zeros((128, WBLOB_TOT), np.float32)
        for nm, part, shp in WBLOB_SPECS:
            off, flat = WBLOB_OFF[nm]
            blob[0:part, off:off + flat] = tensors[nm].reshape(part, flat)
        return blob

    # per-image dictionary tensors
    per_img = []
    for n in range(N_IMG):
        dc = dcp[n]                                       # [64, 256]
        dc_hi = _to_fp32r(dc)
        smat = (np.eye(256, dtype=np.float64)
                - (dc.astype(np.float64).T @ dc.astype(np.float64)) / c).astype(np.float32)
        per_img.append({
            "dc": dc_hi,
            "dc_lo": (dc - dc_hi).astype(np.float32),
            "smat": np.ascontiguousarray(
                smat.reshape(2, 128, 2, 128).transpose(1, 0, 2, 3)),
            "dcT": _split2(np.ascontiguousarray(dc.T)),
        })

    blobs = [build_blob({**shared, **per_img[n]}) for n in range(N_IMG)]
    in_maps = []
    for core in range(N_CORES):
        n, half = core // 2, core % 2
        start = 0 if half == 0 else ODD_START
        unfT = np.ascontiguousarray(unf[n][start:start + R_CORE].T)  # [64, R_CORE]
        unfT_hi = _to_fp32r(unfT)
        in_maps.append({"unfT": unfT_hi,
                        "unfT_lo": (unfT - unfT_hi).astype(np.float32),
                        "wblob": blobs[n], "biases": biases})

    nc = _get_module(c, reps=_reps)
    res = run_bass_kernel_spmd(nc, in_maps, list(range(N_CORES)), trace=False)

    # reassemble full [N, 64, P] outputs (discard overlap region duplicates)
    xw_full = np.zeros((N_IMG, 64, P_ROWS), np.float32)
    wg_full = np.zeros((N_IMG, 64, P_ROWS), np.float32)
    for core in range(N_CORES):
        n, half = core // 2, core % 2
        r = res.results[core]
        keep = (P_ROWS + 1) // 2          # 7321 rows from the even core
        if half == 0:
            xw_full[n, :, :keep] = r["oxw"][:, :keep]
            wg_full[n, :, :keep] = r["owg"][:, :keep]
        else:
            xw_full[n, :, keep:] = r["oxw"][:, keep - ODD_START:]
            wg_full[n, :, keep:] = r["owg"][:, keep - ODD_START:]

    fx = _fold(xw_full, H, W, PATCH)
    fn = _fold(wg_full, H, W, PATCH)
    return (fx / fn)[:, None].astype(np.float32)


# revision 25
# speedup vs baseline: 1.0180x; 1.0055x over previous
"""Trainium2 Bass kernel for nn_DenoisingNet_MLP (8-core data parallel).

Strategy: the per-patch pipeline (threshold MLPs, weight MLP, unrolled ISTA,
reconstruction) runs on device, sharded over the patch dimension: each image's
14641 patch rows are split across 2 cores (4 images x 2 = 8 cores).  The tiny
adaptive-dictionary branch (MLP on [4,112,64] + CBAM) runs on host to produce
each image's dictionary Dc; unfold/fold are host-side data movement.

Device layout is feature-major: activations are [features(part), rows(free)].
All matmuls run as float32r (TF32-like, full PE rate); the ISTA soft-threshold
sign(w)*max(|w|-l,0) is a single custom VectorE op; PSUM accumulates
w = z + y/c - (Dc^T Dc)z/c via identity/Gram matmuls.
"""
import numpy as np

import concourse.bacc as bacc
import concourse.mybir as mybir
import concourse.tile as tile
import concourse.dve_ops as dve_ops
from concourse.dve_spec import Src0, Src1, Zero, maxx, lower, Spec
from concourse.dve_uop import DveOpSpec
from concourse.bass_utils import run_bass_kernel_spmd

dt = mybir.dt
alu = mybir.AluOpType
act_f = mybir.ActivationFunctionType

# ---- problem constants (hardcoded) ----------------------------------------
N_IMG = 4
H = W = 128
PATCH = 8
T_ITER = 5
P_ROWS = 121 * 121          # 14641 patches per image
F = 512                     # max rows per sub-tile (free dim)
SUBS = [512] * 13 + [410, 256]  # per-sub-tile rows (even, >=256 for fp32r)
CS_OFF = [sum(SUBS[:i]) for i in range(len(SUBS))]
R_CORE = sum(SUBS)          # 7424 rows per core
NSUB = len(SUBS)
ODD_START = P_ROWS - R_CORE  # 7217: second core of each image starts here
N_CORES = 8

_module_cache = {}

# weight blob layout: (name, partitions, free shape)
WBLOB_SPECS = [
    ("l1_w", 64, (128,)), ("l2_w", 128, (256,)), ("l3_w", 128, (2, 128)),
    ("pd1_w", 64, (128,)), ("pd2_w", 128, (256,)), ("pd3_w", 128, (2, 128)),
    ("l4x_w", 128, (128,)), ("pd4h_w", 128, (128,)), ("pd4t_w", 128, (128,)),
    ("w1_w", 64, (256,)), ("w2_w", 128, (2, 128)), ("w3_w", 128, (64,)),
    ("dc", 64, (256,)), ("dc_lo", 64, (256,)), ("smat", 128, (2, 2, 128)),
    ("dcT", 128, (2, 64)), ("ident", 128, (128,)),
]
WBLOB_OFF = {}
_off = 0
for _nm, _p, _shp in WBLOB_SPECS:
    _flat = int(np.prod(_shp))
    WBLOB_OFF[_nm] = (_off, _flat)
    _off += _flat
WBLOB_TOT = _off
WBLOB_HOT = 640  # l1_w + l2_w + l3_w: what the first MLP chain needs


# ---- custom DVE op: soft threshold ----------------------------------------
def _soft_ref(in0, in1, s0, s1, imm2):
    return (np.sign(in0) * np.maximum(np.abs(in0) - in1, 0.0)).astype(np.float32)


def _register_soft_op():
    if "SOFT_THRESH_ANT" in dve_ops._SUB_OPCODE_FOR_NAME:
        return next(o for o in dve_ops.OPS if o.name == "SOFT_THRESH_ANT")
    s = (Src0 > Zero) - (Src0 < Zero)
    spec = Spec(body=maxx(s * Src0 - Src1, Zero) * s, reference=_soft_ref)
    shas = {}
    for ver in ("v3", "v4"):
        try:
            u = lower(spec, ver=ver)
            shas[ver] = DveOpSpec(name="SOFT_THRESH_ANT", opcode=31, uops=u,
                                  rd1_en=True).sha(ver)
        except Exception:
            pass
    op = dve_ops.DveOp("SOFT_THRESH_ANT", spec, subdim=False, uops_sha=shas)
    dve_ops.OPS.append(op)
    dve_ops.CUSTOM_DVE_SPECS[op.name] = spec
    dve_ops._SUB_OPCODE_FOR_NAME[op.name] = 31
    return op


# ---- host-side helpers ----------------------------------------------------
def _unfold(x, k, s):
    w = np.lib.stride_tricks.sliding_window_view(x[:, 0], (k, k), axis=(1, 2))
    w = w[:, ::s, ::s]
    n, ho, wo = w.shape[0], w.shape[1], w.shape[2]
    return w.reshape(n, ho * wo, k * k)


def _fold(patches, h, w, k):
    n, d, p = patches.shape
    hp = wp = h - k + 1
    pr = patches.reshape(n, k, k, hp, wp)
    out = np.zeros((n, h, w), np.float32)
    for i in range(k):
        for j in range(k):
            out[:, i:i + hp, j:j + wp] += pr[:, i, j]
    return out


def _mlp(x, p, names):
    for nm in names[:-1]:
        x = np.maximum(x @ p[nm + '_w'] + p[nm + '_b'], 0.0)
    nm = names[-1]
    return x @ p[nm + '_w'] + p[nm + '_b']


def _sigmoid(x):
    return 1.0 / (1.0 + np.exp(-x))


def _cbam(x, p):
    avg = x.mean(axis=(2, 3))
    mx = x.max(axis=(2, 3))
    mlp = lambda v: np.maximum(v @ p['ca1_w'] + p['ca1_b'], 0.0) @ p['ca2_w'] + p['ca2_b']
    ca = _sigmoid(mlp(avg) + mlp(mx))
    x = x * ca[:, :, None, None]
    s = np.stack([x.mean(axis=1), x.max(axis=1)], axis=1)
    sw = p['sa_w'][0]
    pad = np.pad(s, ((0, 0), (0, 0), (3, 3), (3, 3)))
    n = x.shape[0]
    sa = np.zeros((n, 8, 14), np.float32)
    for di in range(7):
        for dj in range(7):
            sa += pad[:, 0, di:di + 8, dj:dj + 14] * sw[0, di, dj]
            sa += pad[:, 1, di:di + 8, dj:dj + 14] * sw[1, di, dj]
    sa = _sigmoid(sa + p['sa_b'][0])
    return x * sa[:, None]


def _small_branch(x, p):
    """Adaptive dictionary branch -> atom-permuted Dc [N, 64, 256]."""
    n = x.shape[0]
    unf1 = _unfold(x, PATCH, PATCH)
    step = unf1.shape[1] // 112
    ue = unf1[:, ::step, :][:, :112, :]
    sdict = _mlp(ue, p, ['ls1', 'ls2', 'ls3', 'ls4'])
    nrm = np.maximum(np.linalg.norm(sdict, axis=-1, keepdims=True), 1e-12)
    sdict = sdict / nrm
    sdict = sdict.transpose(0, 2, 1).reshape(n, 64, 8, 14)
    sdict = _cbam(sdict, p)
    sdict = sdict.reshape(n, 64, 112)
    dc = np.concatenate(
        [np.broadcast_to(np.asarray(p['Dict']), (n, 64, 144)), sdict], axis=-1)
    # atom order: [lam 0:112 | pd 0:16 | pd 16:144]  (so the l4/pd4 heads pack
    # into two clean 128-partition tiles)
    perm = list(range(144, 256)) + list(range(0, 16)) + list(range(16, 144))
    return np.ascontiguousarray(dc[:, :, perm]).astype(np.float32)


def _to_fp32r(x):
    v = np.ascontiguousarray(x, np.float32).view(np.uint32).astype(np.uint64)
    add = 0x7FF + ((v >> 12) & 1)
    return ((v + add) & 0xFFFFF000).astype(np.uint32).view(np.float32)


def _split2(arr):
    """[256, M] -> [128, 2, M] so [:, a, :] is rows a*128 .. a*128+127."""
    m = arr.shape[1]
    return np.ascontiguousarray(arr.reshape(2, 128, m).transpose(1, 0, 2))


# ---- device module --------------------------------------------------------
def _build_module(reps=1):
    soft_op = _register_soft_op()
    nc = bacc.Bacc("TRN2")

    f32, f32r = dt.float32, dt.float32r
    din = {}
    for nm, shp, dty in [
        ("unfT", [64, R_CORE], f32r), ("unfT_lo", [64, R_CORE], f32r),
        ("wblob", [128, WBLOB_TOT], f32r),
        ("biases", [128, 14], f32),
    ]:
        din[nm] = nc.dram_tensor(nm, shp, dty, kind="ExternalInput")
    oxw = nc.dram_tensor("oxw", [64, R_CORE], f32, kind="ExternalOutput")
    owg = nc.dram_tensor("owg", [64, R_CORE], f32, kind="ExternalOutput")

    with tile.TileContext(nc) as tc:
        with tc.tile_pool(name="wts", bufs=1) as wts, \
             tc.tile_pool(name="xin", bufs=4) as xin, \
             tc.tile_pool(name="hbuf", bufs=8) as hbuf, \
             tc.tile_pool(name="h3p", bufs=3) as h3pool, \
             tc.tile_pool(name="zpool", bufs=8) as zpool, \
             tc.tile_pool(name="pair", bufs=4) as pairp, \
             tc.tile_pool(name="outs", bufs=3) as outsp, \
             tc.tile_pool(name="mp", bufs=PSUM_MP, space="PSUM") as mp, \
             tc.tile_pool(name="pp", bufs=PSUM_PP, space="PSUM") as pp:

            # load constants: a hot chunk (first chain's weights) on the HW
            # DGE queue so the first matmul starts ~4us in, the rest in
            # parallel on the gpsimd (SWDGE) queue. Separate tiles so tile
            # dependency tracking doesn't serialize readers on both DMAs.
            wt_hot = wts.tile([128, WBLOB_HOT], f32r, tag="wbhot")
            nc.sync.dma_start(wt_hot[:], din["wblob"][:, 0:WBLOB_HOT])
            wt_cold = wts.tile([128, WBLOB_TOT - WBLOB_HOT], f32r, tag="wbcold")
            nc.gpsimd.dma_start(wt_cold[:], din["wblob"][:, WBLOB_HOT:])
            wsb = {}
            for nm, part, shp in WBLOB_SPECS:
                off, flat = WBLOB_OFF[nm]
                if off < WBLOB_HOT:
                    ap = wt_hot[0:part, off:off + flat]
                else:
                    ap = wt_cold[0:part, off - WBLOB_HOT:off - WBLOB_HOT + flat]
                if len(shp) == 2:
                    ap = ap.rearrange("p (a m) -> p a m", a=shp[0])
                elif len(shp) == 3:
                    ap = ap.rearrange("p (a b m) -> p a b m", a=shp[0], b=shp[1])
                wsb[nm] = ap
            b = wts.tile([128, 14], f32, tag="biases")
            nc.gpsimd.dma_start(b[:], din["biases"][:])
            bcol = lambda j, p=128: b[0:p, j:j + 1]

            # scalar 1/c lives in a [1,1] tile; scale APs must be per-partition,
            # so instead c-folding is done host-side in biases / weights where
            # needed, and 1/c is applied via activation scale=imm below.
            # (cinv input kept for generality; value also baked into scales.)

            def relu_evict(out, psum, bias, eng):
                if eng == "act":
                    nc.scalar.activation(out, psum, act_f.Relu, bias=bias, scale=1.0)
                else:
                    nc.vector.tensor_scalar(out, psum, bias, 0.0, alu.add, alu.max)

            def chain4_gen(x_t, w1, b1, w2, b2a, b2b, w3, b3, tagp, out, eng="act"):
                """64 ->128 ->256 ->128 relu chain; leaves h3 tile in out[tagp]."""
                fs = x_t.shape[-1]
                ps1 = mp.tile([128, fs], f32, tag="mp")
                nc.tensor.matmul(ps1[:], wsb[w1][:], x_t[:], start=True, stop=True)
                h1 = hbuf.tile([128, fs], f32r, tag="h")
                relu_evict(h1[:], ps1[:], b1, eng)
                yield
                ps2a = mp.tile([128, fs], f32, tag="mp")
                nc.tensor.matmul(ps2a[:], wsb[w2][:, 0:128], h1[:], start=True, stop=True)
                h2a = hbuf.tile([128, fs], f32r, tag="h")
                relu_evict(h2a[:], ps2a[:], b2a, eng)
                ps2b = mp.tile([128, fs], f32, tag="mp")
                nc.tensor.matmul(ps2b[:], wsb[w2][:, 128:256], h1[:], start=True, stop=True)
                h2b = hbuf.tile([128, fs], f32r, tag="h")
                relu_evict(h2b[:], ps2b[:], b2b, eng)
                yield
                ps3 = mp.tile([128, fs], f32, tag="mp")
                nc.tensor.matmul(ps3[:], wsb[w3][:, 0, :], h2a[:], start=True, stop=False)
                nc.tensor.matmul(ps3[:], wsb[w3][:, 1, :], h2b[:], start=False, stop=True)
                h3 = h3pool.tile([128, fs], f32r, tag=tagp)
                relu_evict(h3[:], ps3[:], b3, eng)
                out[tagp] = h3

            def front_gen(s, st):
                """MLP chains + thresholds + wg + y + z0 for sub-tile s."""
                fs = SUBS[s]
                cs = slice(CS_OFF[s], CS_OFF[s] + fs)
                st["fs"] = fs
                x_t = xin.tile([64, fs], f32r, tag="x")
                nc.sync.dma_start(x_t[:], din["unfT"][:, cs])
                x_lo = xin.tile([64, fs], f32r, tag="xlo")
                nc.sync.dma_start(x_lo[:], din["unfT_lo"][:, cs])
                yield

                hh = {}
                yield from chain4_gen(x_t, "l1_w", bcol(0), "l2_w", bcol(1), bcol(2),
                                      "l3_w", bcol(3), "h3l", hh)
                yield
                yield from chain4_gen(x_t, "pd1_w", bcol(4), "pd2_w", bcol(5), bcol(6),
                                      "pd3_w", bcol(7), "h3p", hh, eng=PD_ENG)
                yield
                h3l, h3p = hh["h3l"], hh["h3p"]

                l_t_full = pairp.tile([128, 2, F], f32, tag="l")
                l_t = l_t_full[:, :, 0:fs]
                ps_la = mp.tile([128, fs], f32, tag="mp")
                nc.tensor.matmul(ps_la[:], wsb["l4x_w"][:], h3l[:], start=True, stop=False)
                nc.tensor.matmul(ps_la[:], wsb["pd4h_w"][:], h3p[:], start=False, stop=True)
                nc.scalar.activation(l_t[:, 0, :], ps_la[:], act_f.Identity,
                                     bias=bcol(12), scale=CINV)
                ps_lb = mp.tile([128, fs], f32, tag="mp")
                nc.tensor.matmul(ps_lb[:], wsb["pd4t_w"][:], h3p[:], start=True, stop=True)
                nc.scalar.activation(l_t[:, 1, :], ps_lb[:], act_f.Identity,
                                     bias=bcol(13), scale=CINV)
                yield

                psw1a = mp.tile([128, fs], f32, tag="mp")
                nc.tensor.matmul(psw1a[:], wsb["w1_w"][:, 0:128], x_t[:], start=True, stop=True)
                wh1a = hbuf.tile([128, fs], f32r, tag="h")
                nc.scalar.activation(wh1a[:], psw1a[:], act_f.Relu, bias=bcol(8), scale=1.0)
                psw1b = mp.tile([128, fs], f32, tag="mp")
                nc.tensor.matmul(psw1b[:], wsb["w1_w"][:, 128:256], x_t[:], start=True, stop=True)
                wh1b = hbuf.tile([128, fs], f32r, tag="h")
                relu_evict(wh1b[:], psw1b[:], bcol(9), W_ENG)
                yield
                psw2 = mp.tile([128, fs], f32, tag="mp")
                nc.tensor.matmul(psw2[:], wsb["w2_w"][:, 0, :], wh1a[:], start=True, stop=False)
                nc.tensor.matmul(psw2[:], wsb["w2_w"][:, 1, :], wh1b[:], start=False, stop=True)
                wh2 = hbuf.tile([128, fs], f32r, tag="h")
                relu_evict(wh2[:], psw2[:], bcol(10), W_ENG)
                psw3 = mp.tile([64, fs], f32, tag="mp")
                nc.tensor.matmul(psw3[:], wsb["w3_w"][:], wh2[:], start=True, stop=True)
                wg = outsp.tile([64, fs], f32, tag="wg")
                nc.scalar.activation(wg[:], psw3[:], act_f.Sigmoid,
                                     bias=bcol(11, 64), scale=1.0)
                yield

                ps_y_full = pp.tile([128, 2, F], f32, tag="pp")
                ps_y = ps_y_full[:, :, 0:fs]
                for k in (0, 1):
                    ck = slice(k * 128, (k + 1) * 128)
                    nc.tensor.matmul(ps_y[:, k, :], wsb["dc"][:, ck], x_t[:], start=True, stop=False)
                    nc.tensor.matmul(ps_y[:, k, :], wsb["dc"][:, ck], x_lo[:], start=False, stop=False)
                    nc.tensor.matmul(ps_y[:, k, :], wsb["dc_lo"][:, ck], x_t[:], start=False, stop=True)
                yc_full = pairp.tile([128, 2, F], f32r, tag="yc")
                yc = yc_full[:, :, 0:fs]
                nc.scalar.mul(yc[:], ps_y[:], CINV)
                z_full = zpool.tile([128, 2, F], f32r, tag="z")
                z = z_full[:, :, 0:fs]
                nc.vector._custom_dve(soft_op, out=z[:], in0=ps_y[:], in1=l_t[:])
                st.update({"s": s, "z": z, "yc": yc, "l": l_t, "wg": wg})

            def emit_iter_gen(st):
                fs = st["fs"]
                ps_w_full = pp.tile([128, 2, F], f32, tag="pp")
                ps_w = ps_w_full[:, :, 0:fs]
                z, yc = st["z"], st["yc"]
                for k in (0, 1):
                    nc.tensor.matmul(ps_w[:, k, :], wsb["ident"][:],
                                     yc[:, k, :], start=True, stop=False)
                    nc.tensor.matmul(ps_w[:, k, :], wsb["smat"][:, 0, k, :],
                                     z[:, 0, :], start=False, stop=False)
                    nc.tensor.matmul(ps_w[:, k, :], wsb["smat"][:, 1, k, :],
                                     z[:, 1, :], start=False, stop=True)
                    if k == 0:
                        yield
                zn_full = zpool.tile([128, 2, F], f32r, tag="z")
                zn = zn_full[:, :, 0:fs]
                nc.vector._custom_dve(soft_op, out=zn[:], in0=ps_w[:], in1=st["l"])
                st["z"] = zn

            def emit_back(st):
                s, z, wg, fs = st["s"], st["z"], st["wg"], st["fs"]
                cs = slice(CS_OFF[s], CS_OFF[s] + fs)
                ps_xp = mp.tile([64, fs], f32, tag="mp")
                nc.tensor.matmul(ps_xp[:], wsb["dcT"][:, 0, :], z[:, 0, :], start=True, stop=False)
                nc.tensor.matmul(ps_xp[:], wsb["dcT"][:, 1, :], z[:, 1, :], start=False, stop=True)
                xpc = outsp.tile([64, fs], f32, tag="xpc")
                nc.vector.tensor_scalar(xpc[:], ps_xp[:], 0.0, 1.0, alu.max, alu.min)
                xw = outsp.tile([64, fs], f32, tag="xw")
                nc.vector.tensor_tensor(xw[:], xpc[:], wg[:], alu.mult)
                nc.sync.dma_start(oxw[:, cs], xw[:])
                nc.sync.dma_start(owg[:, cs], wg[:])

            def ista_gen(st):
                for _t in range(T_ITER):
                    yield from emit_iter_gen(st)
                    yield
                emit_back(st)

            # software pipeline: front(s) interleaved with the pending ISTA
            # chains of previous sub-tiles so PE stays dense (HAM warm)
            # through the ACT-gated MLP chain stages.
            for _rep in range(reps):
                from collections import deque
                pend = deque()
                rr = 0
                for s in range(NSUB):
                    st = {}
                    fg = front_gen(s, st)
                    k = 0
                    for _ in fg:
                        k += 1
                        if pend and k % ILV == 0:
                            g = pend[rr % len(pend)]
                            if next(g, "done") == "done":
                                pend.remove(g)
                            rr += 1
                    while len(pend) >= PIPE_DEPTH:
                        g = pend[rr % len(pend)]
                        if next(g, "done") == "done":
                            pend.remove(g)
                        rr += 1
                    pend.append(ista_gen(st))
                while pend:
                    g = pend[rr % len(pend)]
                    if next(g, "done") == "done":
                        pend.remove(g)
                    rr += 1

    nc.finalize()
    return nc


CINV = None  # set before _build_module is called (compile-time constant)


def _get_module(c, reps=1):
    global CINV
    key = (round(1.0 / c, 10), reps)
    if key not in _module_cache:
        CINV = 1.0 / c
        _module_cache[key] = _build_module(reps=reps)
    return _module_cache[key]


# ---- public entry ----------------------------------------------------------
def kernel(x, params, _reps=1):
    x = np.asarray(x, np.float32)
    p = {k: np.asarray(v, np.float32) for k, v in params.items()}
    c = float(p['c'])

    dcp = _small_branch(x, p)                   # [N, 64, 256] permuted atoms
    unf = _unfold(x, PATCH, 1)                  # [N, P, 64]

    # shared (replicated) tensors
    l4x_w = np.concatenate([p['l4_w'], np.zeros((128, 16), np.float32)], axis=1)
    pd4h_w = np.concatenate([np.zeros((128, 112), np.float32), p['pd4_w'][:, :16]], axis=1)
    pd4t_w = np.ascontiguousarray(p['pd4_w'][:, 16:144])
    biases = np.zeros((128, 14), np.float32)
    biases[:, 0] = p['l1_b']
    biases[:, 1] = p['l2_b'][:128]
    biases[:, 2] = p['l2_b'][128:]
    biases[:, 3] = p['l3_b']
    biases[:, 4] = p['pd1_b']
    biases[:, 5] = p['pd2_b'][:128]
    biases[:, 6] = p['pd2_b'][128:]
    biases[:, 7] = p['pd3_b']
    biases[:, 8] = p['w1_b'][:128]
    biases[:, 9] = p['w1_b'][128:]
    biases[:, 10] = p['w2_b']
    biases[0:64, 11] = p['w3_b']
    biases[:, 12] = np.concatenate([p['l4_b'][:112], p['pd4_b'][:16]]) / c
    biases[:, 13] = p['pd4_b'][16:144] / c

    shared = {
        "l1_w": p['l1_w'], "l2_w": p['l2_w'], "l3_w": _split2(p['l3_w']),
        "pd1_w": p['pd1_w'], "pd2_w": p['pd2_w'], "pd3_w": _split2(p['pd3_w']),
        "l4x_w": l4x_w, "pd4h_w": pd4h_w, "pd4t_w": pd4t_w,
        "w1_w": p['w1_w'], "w2_w": _split2(p['w2_w']),
        "w3_w": p['w3_w'],
        "ident": np.eye(128, dtype=np.float32),
    }

    def build_blob(tensors):
        blob = np.zeros((128, WBLOB_TOT), np.float32)
        for nm, part, shp in WBLOB_SPECS:
            off, flat = WBLOB_OFF[nm]
            blob[0:part, off:off + flat] = tensors[nm].reshape(part, flat)
        return blob

    # per-image dictionary tensors
    per_img = []
    for n in range(N_IMG):
        dc = dcp[n]                                       # [64, 256]
        dc_hi = _to_fp32r(dc)
        smat = (np.eye(256, dtype=np.float64)
                - (dc.astype(np.float64).T @ dc.astype(np.float64)) / c).astype(np.float32)
        per_img.append({
            "dc": dc_hi,
            "dc_lo": (dc - dc_hi).astype(np.float32),
            "smat": np.ascontiguousarray(
                smat.reshape(2, 128, 2, 128).transpose(1, 0, 2, 3)),
            "dcT": _split2(np.ascontiguousarray(dc.T)),
        })

    blobs = [build_blob({**shared, **per_img[n]}) for n in range(N_IMG)]
    in_maps = []
    for core in range(N_CORES):
        n, half = core // 2, core % 2
        start = 0 if half == 0 else ODD_START
        unfT = np.ascontiguousarray(unf[n][start:start + R_CORE].T)  # [64, R_CORE]
        unfT_hi = _to_fp32r(unfT)
        in_maps.append({"unfT": unfT_hi,
                        "unfT_lo": (unfT - unfT_hi).astype(np.float32),
                        "wblob": blobs[n], "biases": biases})

    nc = _get_module(c, reps=_reps)
    res = run_bass_kernel_spmd(nc, in_maps, list(range(N_CORES)), trace=False)

    # reassemble full [N, 64, P] outputs (discard overlap region duplicates)
    xw_full = np.zeros((N_IMG, 64, P_ROWS), np.float32)
    wg_full = np.zeros((N_IMG, 64, P_ROWS), np.float32)
    for core in range(N_CORES):
        n, half = core // 2, core % 2
        r = res.results[core]
        keep = (P_ROWS + 1) // 2          # 7321 rows from the even core
        if half == 0:
            xw_full[n, :, :keep] = r["oxw"][:, :keep]
            wg_full[n, :, :keep] = r["owg"][:, :keep]
        else:
            xw_full[n, :, keep:] = r["oxw"][:, keep - ODD_START:]
            wg_full[n, :, keep:] = r["owg"][:, keep - ODD_START:]

    fx = _fold(xw_full, H, W, PATCH)
    fn = _fold(wg_full, H, W, PATCH)
    return (fx / fn)[:, None].astype(np.float32)
